# revision 36
# baseline (speedup 1.0000x reference)
"""Trainium2 kernel for nn_Loss_HF_86079734546730.

Strategy (8 NeuronCores, SPMD, no collectives):
  - Shard the two [64,3,512,512] inputs spatially over H: core k gets raw
    rows [64k, 64k+64) => shard [64, 3, 64, 512] per tensor (~25 MiB each).
  - DMA: 12 large HWDGE loads per core (one per tensor x channel x
    half-batch-group, ~4.2 MB each) issued on the sync queue -- amortizes
    the ~2us per-DMA completion latency that dominates with small DMAs.
  - Band build on PE only (no DVE preprocessing): per (tensor, channel,
    batch-pair j) tile [128=(2b x 64h), 512w] in fp32r, two matmuls per
    128-wb chunk accumulate the even-w and odd-w halves against constant
    Haar-combination matrices W_e / W_o [128, 256]. The stationary operand
    is the data (its free dim wb becomes the PSUM partition dim => output
    arrives transposed to [wb, feature-cols], exactly what the Gram stage
    needs), and the W columns encode the vertical Haar combos for all 3
    bands. fp32r runs at bf16 speed when the moving dim is >= 256 (W is
    zero-padded 192->256 for that reason).
  - ACT/DVE (alternating): copy PSUM [128, 0:192] -> bf16 band buffer,
    hb-major layout: col = hb*194 + (6j + b'*3 + band); col 192 of each
    194-block is a constant 1.0 (gives feature sums in the Gram).
  - PE: per (tensor, channel): Gram of the band buffer accumulated in PSUM
    fp32 over all 64 (wbc, hb) spatial chunks. Symmetric trick: chunk0
    computes rows 0:128 x cols 0:193, chunk1 only rows/cols 128:193; the
    host mirrors the missing block.
  - Host (float64): sum partial Grams over cores, rebuild per-(b,c,band)
    mean/std, expand the normalized-feature Gram algebraically, cosine-sim,
    softmax, KL.
"""

import numpy as np

B, C, H, W = 64, 3, 512, 512
NCORES = 8
HSH = H // NCORES          # 64 raw rows per core
NJ = B // 2                # 32 batch pairs
HB = HSH // 2              # 32 band rows per core
WB = W // 2                # 256 band cols
CCB = 194                  # band-buffer cols per hb block: 192 feat + 1 ones + 1 pad
EPS_STD = 1e-5
EPS_COS = 1e-8
EPS_P = 1e-8

_CACHE = {}


def _make_w():
    """Constant matrices for the band-build matmuls, [128, 512] fp32.

    Rows: p = (two in 2) x (h in 64)  [partition layout of the raw tiles]
    Cols: ew*256 + hb*6 + two'*3 + band   (band: 0=lh, 1=hh, 2=hl),
    cols 192:256 of each half are zero padding (keeps the fp32r moving dim
    at 256 for full rate).
      W[two*64 + 2*hb + eh, ew*256 + hb*6 + two*3 + band] = s(band, eh, ew)
      s(lh) = 0.5*(1-2*ew); s(hh) = -0.5 if eh==ew else 0.5; s(hl) = 0.5*(1-2*eh)
    """
    w = np.zeros((128, 512), np.float32)
    for two in range(2):
        for hb in range(HB):
            for eh in range(2):
                r = two * 64 + 2 * hb + eh
                for ew in range(2):
                    base = ew * 256 + hb * 6 + two * 3
                    w[r, base + 0] = 0.5 * (1 - 2 * ew)
                    w[r, base + 1] = -0.5 if eh == ew else 0.5
                    w[r, base + 2] = 0.5 * (1 - 2 * eh)
    return w


def _col_batch_map():
    """Global feature g = 6j + two'*3 + band  ->  batch 2j + two'."""
    cb = np.zeros(192, np.int64)
    for j in range(NJ):
        for bp in range(2):
            for band in range(3):
                cb[6 * j + bp * 3 + band] = 2 * j + bp
    return cb


def _build_nc():
    import concourse.bass as bass
    import concourse.mybir as mybir
    import concourse.tile as tile
    from concourse import bacc

    f32 = mybir.dt.float32
    f32r = mybir.dt.float32r
    bf16 = mybir.dt.bfloat16

    nc = bacc.Bacc()
    za = nc.declare_dram_parameter("za", [B, C, HSH, W], f32r, isOutput=False)
    zs = nc.declare_dram_parameter("zs", [B, C, HSH, W], f32r, isOutput=False)
    wmat = nc.declare_dram_parameter("wmat", [128, 512], f32r, isOutput=False)
    gout = nc.declare_dram_parameter("G", [2, C, 193, 193], f32, isOutput=True)
    zz = [za, zs]

    NBCOL = 2 * HB * CCB  # both wbc chunks in one buffer: 12416

    with tile.TileContext(nc) as tc:
        with (
            tc.tile_pool(name="wconst", bufs=1) as w_pool,
            tc.tile_pool(name="raw", bufs=5) as raw_pool,
            tc.tile_pool(name="raw0", bufs=2) as raw0_pool,
            tc.tile_pool(name="bands", bufs=2) as band_pool,
            tc.tile_pool(name="stage", bufs=4) as stage_pool,
            tc.tile_pool(name="pband", bufs=6, space="PSUM") as pb_pool,
            tc.tile_pool(name="pgram", bufs=1, space="PSUM") as pg_pool,
        ):
            w_t = w_pool.tile([128, 512], f32r, tag="wmat")
            nc.sync.dma_start(w_t[:], wmat[:])

            # the two load halves go through the two HWDGE rings (disjoint
            # even/odd SDMA engine sets -> they transfer concurrently)
            loaders = [nc.sync, nc.scalar]

            for c in range(C):
                bbs = {}
                for t in range(2):
                    bb = band_pool.tile([128, NBCOL], bf16, tag=f"bb{t}")
                    bb4 = bb[:].rearrange(
                        "p (wbc hb cc) -> p wbc hb cc", hb=HB, cc=CCB
                    )
                    nc.gpsimd.memset(bb4[:, :, :, 192], 1.0)
                    bbs[t] = bb

                for t in range(2):
                    # 8-j raw tiles keep the load flow continuous (fine-
                    # grained buffer release) so the PE never starves at group
                    # boundaries. The very first and last tiles are 4-j tiles
                    # from a dedicated pool (uniform sizes per pool tag): the
                    # first matmuls wait on a half-size DMA, and the final
                    # gram chases a half-size tail load.
                    if c == 0 and t == 0:
                        jsplit = [(0, 4), (4, 8), (8, 16), (16, 24), (24, 32)]
                    elif c == C - 1 and t == 1:
                        jsplit = [(0, 8), (8, 16), (16, 24), (24, 28), (28, 32)]
                    else:
                        jsplit = [(0, 8), (8, 16), (16, 24), (24, 32)]
                    for j0, j1 in jsplit:
                        nj = j1 - j0
                        pool = raw_pool if nj == 8 else raw0_pool
                        raw = pool.tile(
                            [128, nj * 512], f32r, tag="raw" if nj == 8 else "raw0"
                        )
                        # DMA APs are limited to 3 dims, so split the load by
                        # the batch-within-pair dim; the two halves go to
                        # different HWDGE queues (disjoint even/odd SDMA
                        # engine sets) so they transfer concurrently. Both
                        # HWDGE engines (sync/scalar) carry nothing else, so
                        # ring blocking never stalls compute issue.
                        src = zz[t][2 * j0 : 2 * j1, c].rearrange(
                            "(j two) h w -> two h j w", two=2
                        )
                        for two in range(2):
                            loaders[two].dma_start(
                                raw[64 * two : 64 * (two + 1), :], src[two]
                            )
                        for jl in range(nj):
                            j = j0 + jl
                            rw = raw[:, 512 * jl : 512 * (jl + 1)].rearrange(
                                "p (w two) -> p w two", two=2
                            )
                            # both wbc chunks in one PSUM bank tile; the two
                            # accumulation groups are strictly ordered so the
                            # bank-wide has_written clear of the second group
                            # cannot corrupt the first
                            # both wbc chunks in one PSUM bank tile; the two
                            # accumulation groups are strictly ordered so the
                            # bank-wide has_written clear of the second group
                            # cannot corrupt the first
                            pband = pb_pool.tile([128, 512], f32, tag="pband")
                            for wbc in range(2):
                                for ew in range(2):
                                    nc.tensor.matmul(
                                        pband[:, 256 * wbc : 256 * (wbc + 1)],
                                        rw[:, 128 * wbc : 128 * (wbc + 1), ew],
                                        w_t[:, 256 * ew : 256 * ew + 256],
                                        start=(ew == 0),
                                        stop=(ew == 1),
                                    )
                            # single fused DVE copy (PSUM fp32 -> bf16 bands).
                            # ACT must NOT take copies: it carries a HWDGE
                            # load ring, and ring-blocked DMA instructions
                            # would stall queued ACT copies (measured 25%+
                            # regression when tried).
                            src_v = pband[:].rearrange(
                                "p (wbc x) -> p wbc x", x=256
                            )[:, :, 0:192].rearrange(
                                "p wbc (hb l) -> p wbc hb l", l=6
                            )
                            dst_v = bbs[t][:].rearrange(
                                "p (wbc hb cc) -> p wbc hb cc", hb=HB, cc=CCB
                            )[:, :, :, 6 * j : 6 * j + 6]
                            nc.vector.tensor_copy(dst_v, src_v)

                    # gram(t) right after bands(t): fills the PE while the
                    # next group's loads arrive, and shortens the end tail
                    bb4 = bbs[t][:].rearrange(
                        "p (wbc hb cc) -> p wbc hb cc", hb=HB, cc=CCB
                    )
                    # chunk0: rows 0:128 x cols 0:193; chunk1: rows/cols 128:193
                    for chunk in range(2):
                        if chunk == 0:
                            rows, cs, ce, ms, me = 128, 0, 128, 0, 193
                        else:
                            rows, cs, ce, ms, me = 65, 128, 193, 128, 193
                        pg = pg_pool.tile([rows, me - ms], f32, tag=f"pg{chunk}")
                        for wbc in range(2):
                            for hb in range(HB):
                                nc.tensor.matmul(
                                    pg[:],
                                    bb4[:, wbc, hb, cs:ce],
                                    bb4[:, wbc, hb, ms:me],
                                    start=(wbc == 0 and hb == 0),
                                    stop=(wbc == 1 and hb == HB - 1),
                                )
                        st = stage_pool.tile([rows, me - ms], f32, tag=f"st{chunk}")
                        nc.vector.tensor_copy(st[:], pg[:])
                        # store via SWDGE (gpsimd is otherwise idle) so it
                        # never stalls the two HWDGE load rings
                        nc.gpsimd.dma_start(
                            gout[t, c, cs : cs + rows, ms:me], st[:]
                        )
    if not nc.is_finalized():
        nc.finalize()
    return nc


def _get_nc():
    if "nc" not in _CACHE:
        _CACHE["nc"] = _build_nc()
    return _CACHE["nc"]


def _host_finish(g_parts):
    """g_parts: list of per-core G arrays [2,3,193,193] (fp32). Returns KL."""
    g = np.zeros((2, C, 193, 193), np.float64)
    for arr in g_parts:
        g += np.asarray(arr, np.float64)
    # mirror the symmetric block the kernel skipped
    g[:, :, 128:, :128] = np.swapaxes(g[:, :, :128, 128:], -1, -2)

    cb = _col_batch_map()
    S = float(g[0, 0, 192, 192])

    P = np.zeros((2, B, B), np.float64)
    Bm = np.zeros((192, B), np.float64)
    Bm[np.arange(192), cb] = 1.0
    for t in range(2):
        for c in range(C):
            M = g[t, c, :192, :192]
            Tv = g[t, c, 192, :192]
            mu = Tv / S
            var = (np.diag(M) - Tv * Tv / S) / (S - 1.0)
            sig = np.sqrt(np.maximum(var, 0.0))
            alpha = 1.0 / (3.0 * (sig + EPS_STD))
            Mc = M - np.outer(mu, Tv) - np.outer(Tv, mu) + S * np.outer(mu, mu)
            Ms = (alpha[:, None] * Mc) * alpha[None, :]
            P[t] += Bm.T @ Ms @ Bm

    sims = []
    for t in range(2):
        r = np.sqrt(np.maximum(np.diag(P[t]), 0.0))
        rc = np.maximum(r, EPS_COS)
        sims.append(P[t] / np.outer(rc, rc))

    def softmax_offdiag(sim):
        m = sim.copy()
        np.fill_diagonal(m, -np.inf)
        mx = m.max(axis=1, keepdims=True)
        e = np.exp(m - mx)
        return e / e.sum(axis=1, keepdims=True)

    p_ada = softmax_offdiag(sims[0]) + EPS_P
    p_sou = softmax_offdiag(sims[1]) + EPS_P
    kl = np.sum(p_sou * (np.log(p_sou) - np.log(p_ada))) / B
    return np.float32(kl)


def _make_in_maps(z_ada, z_sou):
    wmat = _make_w()
    in_maps = []
    for k in range(NCORES):
        sl = slice(HSH * k, HSH * (k + 1))
        in_maps.append(
            {
                "za": np.ascontiguousarray(z_ada[:, :, sl, :]),
                "zs": np.ascontiguousarray(z_sou[:, :, sl, :]),
                "wmat": wmat,
            }
        )
    return in_maps


def kernel(z_ada, z_sou):
    from concourse.bass_utils import run_bass_kernel_spmd

    z_ada = np.asarray(z_ada, np.float32)
    z_sou = np.asarray(z_sou, np.float32)
    in_maps = _make_in_maps(z_ada, z_sou)
    nc = _get_nc()
    res = run_bass_kernel_spmd(nc, in_maps, list(range(NCORES)))
    g_parts = [res.results[k]["G"] for k in range(NCORES)]
    return _host_finish(g_parts)


# revision 37
# speedup vs baseline: 1.0163x; 1.0163x over previous
"""Trainium2 kernel for nn_Loss_HF_86079734546730.

Strategy (8 NeuronCores, SPMD, no collectives):
  - Shard the two [64,3,512,512] inputs spatially over H: core k gets raw
    rows [64k, 64k+64) => shard [64, 3, 64, 512] per tensor (~25 MiB each).
  - DMA: 12 large HWDGE loads per core (one per tensor x channel x
    half-batch-group, ~4.2 MB each) issued on the sync queue -- amortizes
    the ~2us per-DMA completion latency that dominates with small DMAs.
  - Band build on PE only (no DVE preprocessing): per (tensor, channel,
    batch-pair j) tile [128=(2b x 64h), 512w] in fp32r, two matmuls per
    128-wb chunk accumulate the even-w and odd-w halves against constant
    Haar-combination matrices W_e / W_o [128, 256]. The stationary operand
    is the data (its free dim wb becomes the PSUM partition dim => output
    arrives transposed to [wb, feature-cols], exactly what the Gram stage
    needs), and the W columns encode the vertical Haar combos for all 3
    bands. fp32r runs at bf16 speed when the moving dim is >= 256 (W is
    zero-padded 192->256 for that reason).
  - ACT/DVE (alternating): copy PSUM [128, 0:192] -> bf16 band buffer,
    hb-major layout: col = hb*194 + (6j + b'*3 + band); col 192 of each
    194-block is a constant 1.0 (gives feature sums in the Gram).
  - PE: per (tensor, channel): Gram of the band buffer accumulated in PSUM
    fp32 over all 64 (wbc, hb) spatial chunks. Symmetric trick: chunk0
    computes rows 0:128 x cols 0:193, chunk1 only rows/cols 128:193; the
    host mirrors the missing block.
  - Host (float64): sum partial Grams over cores, rebuild per-(b,c,band)
    mean/std, expand the normalized-feature Gram algebraically, cosine-sim,
    softmax, KL.
"""

import numpy as np

B, C, H, W = 64, 3, 512, 512
NCORES = 8
HSH = H // NCORES          # 64 raw rows per core
NJ = B // 2                # 32 batch pairs
HB = HSH // 2              # 32 band rows per core
WB = W // 2                # 256 band cols
CCB = 194                  # band-buffer cols per hb block: 192 feat + 1 ones + 1 pad
EPS_STD = 1e-5
EPS_COS = 1e-8
EPS_P = 1e-8

_CACHE = {}


def _make_w():
    """Constant matrices for the band-build matmuls, [128, 512] fp32.

    Rows: p = (two in 2) x (h in 64)  [partition layout of the raw tiles]
    Cols: ew*256 + hb*6 + two'*3 + band   (band: 0=lh, 1=hh, 2=hl),
    cols 192:256 of each half are zero padding (keeps the fp32r moving dim
    at 256 for full rate).
      W[two*64 + 2*hb + eh, ew*256 + hb*6 + two*3 + band] = s(band, eh, ew)
      s(lh) = 0.5*(1-2*ew); s(hh) = -0.5 if eh==ew else 0.5; s(hl) = 0.5*(1-2*eh)
    """
    w = np.zeros((128, 512), np.float32)
    for two in range(2):
        for hb in range(HB):
            for eh in range(2):
                r = two * 64 + 2 * hb + eh
                for ew in range(2):
                    base = ew * 256 + hb * 6 + two * 3
                    w[r, base + 0] = 0.5 * (1 - 2 * ew)
                    w[r, base + 1] = -0.5 if eh == ew else 0.5
                    w[r, base + 2] = 0.5 * (1 - 2 * eh)
    return w


def _col_batch_map():
    """Global feature g = 6j + two'*3 + band  ->  batch 2j + two'."""
    cb = np.zeros(192, np.int64)
    for j in range(NJ):
        for bp in range(2):
            for band in range(3):
                cb[6 * j + bp * 3 + band] = 2 * j + bp
    return cb


def _build_nc():
    import concourse.bass as bass
    import concourse.mybir as mybir
    import concourse.tile as tile
    from concourse import bacc

    f32 = mybir.dt.float32
    f32r = mybir.dt.float32r
    bf16 = mybir.dt.bfloat16

    nc = bacc.Bacc()
    za = nc.declare_dram_parameter("za", [B, C, HSH, W], f32r, isOutput=False)
    zs = nc.declare_dram_parameter("zs", [B, C, HSH, W], f32r, isOutput=False)
    wmat = nc.declare_dram_parameter("wmat", [128, 512], f32r, isOutput=False)
    gout = nc.declare_dram_parameter("G", [2, C, 193, 193], f32, isOutput=True)
    zz = [za, zs]

    NBCOL = 2 * HB * CCB  # both wbc chunks in one buffer: 12416

    with tile.TileContext(nc) as tc:
        with (
            tc.tile_pool(name="wconst", bufs=1) as w_pool,
            tc.tile_pool(name="raw", bufs=6) as raw_pool,
            tc.tile_pool(name="bands", bufs=2) as band_pool,
            tc.tile_pool(name="stage", bufs=4) as stage_pool,
            tc.tile_pool(name="pband", bufs=6, space="PSUM") as pb_pool,
            tc.tile_pool(name="pgram", bufs=1, space="PSUM") as pg_pool,
        ):
            w_t = w_pool.tile([128, 512], f32r, tag="wmat")
            nc.sync.dma_start(w_t[:], wmat[:])

            # the two load halves go through the two HWDGE rings (disjoint
            # even/odd SDMA engine sets -> they transfer concurrently)
            loaders = [nc.sync, nc.scalar]

            for c in range(C):
                bbs = {}
                for t in range(2):
                    bb = band_pool.tile([128, NBCOL], bf16, tag=f"bb{t}")
                    bb4 = bb[:].rearrange(
                        "p (wbc hb cc) -> p wbc hb cc", hb=HB, cc=CCB
                    )
                    nc.gpsimd.memset(bb4[:, :, :, 192], 1.0)
                    bbs[t] = bb

                for t in range(2):
                    # 8-j raw tiles with 6 buffers keep the load flow
                    # continuous (fine-grained buffer release) so the PE never
                    # starves at group boundaries.
                    for j0, j1 in [(0, 8), (8, 16), (16, 24), (24, 32)]:
                        nj = j1 - j0
                        raw = raw_pool.tile([128, nj * 512], f32r, tag="raw")
                        # DMA APs are limited to 3 dims, so split the load by
                        # the batch-within-pair dim; the two halves go to
                        # different HWDGE queues (disjoint even/odd SDMA
                        # engine sets) so they transfer concurrently. Both
                        # HWDGE engines (sync/scalar) carry nothing else, so
                        # ring blocking never stalls compute issue.
                        src = zz[t][2 * j0 : 2 * j1, c].rearrange(
                            "(j two) h w -> two h j w", two=2
                        )
                        for two in range(2):
                            loaders[two].dma_start(
                                raw[64 * two : 64 * (two + 1), :], src[two]
                            )
                        for jl in range(nj):
                            j = j0 + jl
                            rw = raw[:, 512 * jl : 512 * (jl + 1)].rearrange(
                                "p (w two) -> p w two", two=2
                            )
                            # both wbc chunks in one PSUM bank tile; the two
                            # accumulation groups are strictly ordered so the
                            # bank-wide has_written clear of the second group
                            # cannot corrupt the first
                            # both wbc chunks in one PSUM bank tile; the two
                            # accumulation groups are strictly ordered so the
                            # bank-wide has_written clear of the second group
                            # cannot corrupt the first
                            pband = pb_pool.tile([128, 512], f32, tag="pband")
                            for wbc in range(2):
                                for ew in range(2):
                                    nc.tensor.matmul(
                                        pband[:, 256 * wbc : 256 * (wbc + 1)],
                                        rw[:, 128 * wbc : 128 * (wbc + 1), ew],
                                        w_t[:, 256 * ew : 256 * ew + 256],
                                        start=(ew == 0),
                                        stop=(ew == 1),
                                    )
                            # single fused DVE copy (PSUM fp32 -> bf16 bands).
                            # ACT must NOT take copies: it carries a HWDGE
                            # load ring, and ring-blocked DMA instructions
                            # would stall queued ACT copies (measured 25%+
                            # regression when tried).
                            src_v = pband[:].rearrange(
                                "p (wbc x) -> p wbc x", x=256
                            )[:, :, 0:192].rearrange(
                                "p wbc (hb l) -> p wbc hb l", l=6
                            )
                            dst_v = bbs[t][:].rearrange(
                                "p (wbc hb cc) -> p wbc hb cc", hb=HB, cc=CCB
                            )[:, :, :, 6 * j : 6 * j + 6]
                            nc.vector.tensor_copy(dst_v, src_v)

                    # gram(t) right after bands(t): fills the PE while the
                    # next group's loads arrive, and shortens the end tail
                    bb4 = bbs[t][:].rearrange(
                        "p (wbc hb cc) -> p wbc hb cc", hb=HB, cc=CCB
                    )
                    # chunk0: rows 0:128 x cols 0:193; chunk1: rows/cols 128:193
                    for chunk in range(2):
                        if chunk == 0:
                            rows, cs, ce, ms, me = 128, 0, 128, 0, 193
                        else:
                            rows, cs, ce, ms, me = 65, 128, 193, 128, 193
                        pg = pg_pool.tile([rows, me - ms], f32, tag=f"pg{chunk}")
                        for wbc in range(2):
                            for hb in range(HB):
                                nc.tensor.matmul(
                                    pg[:],
                                    bb4[:, wbc, hb, cs:ce],
                                    bb4[:, wbc, hb, ms:me],
                                    start=(wbc == 0 and hb == 0),
                                    stop=(wbc == 1 and hb == HB - 1),
                                )
                        st = stage_pool.tile([rows, me - ms], f32, tag=f"st{chunk}")
                        nc.vector.tensor_copy(st[:], pg[:])
                        # store via SWDGE (gpsimd is otherwise idle) so it
                        # never stalls the two HWDGE load rings
                        nc.gpsimd.dma_start(
                            gout[t, c, cs : cs + rows, ms:me], st[:]
                        )
    if not nc.is_finalized():
        nc.finalize()
    return nc


def _get_nc():
    if "nc" not in _CACHE:
        _CACHE["nc"] = _build_nc()
    return _CACHE["nc"]


def _host_finish(g_parts):
    """g_parts: list of per-core G arrays [2,3,193,193] (fp32). Returns KL."""
    g = np.zeros((2, C, 193, 193), np.float64)
    for arr in g_parts:
        g += np.asarray(arr, np.float64)
    # mirror the symmetric block the kernel skipped
    g[:, :, 128:, :128] = np.swapaxes(g[:, :, :128, 128:], -1, -2)

    cb = _col_batch_map()
    S = float(g[0, 0, 192, 192])

    P = np.zeros((2, B, B), np.float64)
    Bm = np.zeros((192, B), np.float64)
    Bm[np.arange(192), cb] = 1.0
    for t in range(2):
        for c in range(C):
            M = g[t, c, :192, :192]
            Tv = g[t, c, 192, :192]
            mu = Tv / S
            var = (np.diag(M) - Tv * Tv / S) / (S - 1.0)
            sig = np.sqrt(np.maximum(var, 0.0))
            alpha = 1.0 / (3.0 * (sig + EPS_STD))
            Mc = M - np.outer(mu, Tv) - np.outer(Tv, mu) + S * np.outer(mu, mu)
            Ms = (alpha[:, None] * Mc) * alpha[None, :]
            P[t] += Bm.T @ Ms @ Bm

    sims = []
    for t in range(2):
        r = np.sqrt(np.maximum(np.diag(P[t]), 0.0))
        rc = np.maximum(r, EPS_COS)
        sims.append(P[t] / np.outer(rc, rc))

    def softmax_offdiag(sim):
        m = sim.copy()
        np.fill_diagonal(m, -np.inf)
        mx = m.max(axis=1, keepdims=True)
        e = np.exp(m - mx)
        return e / e.sum(axis=1, keepdims=True)

    p_ada = softmax_offdiag(sims[0]) + EPS_P
    p_sou = softmax_offdiag(sims[1]) + EPS_P
    kl = np.sum(p_sou * (np.log(p_sou) - np.log(p_ada))) / B
    return np.float32(kl)


def _make_in_maps(z_ada, z_sou):
    wmat = _make_w()
    in_maps = []
    for k in range(NCORES):
        sl = slice(HSH * k, HSH * (k + 1))
        in_maps.append(
            {
                "za": np.ascontiguousarray(z_ada[:, :, sl, :]),
                "zs": np.ascontiguousarray(z_sou[:, :, sl, :]),
                "wmat": wmat,
            }
        )
    return in_maps


def kernel(z_ada, z_sou):
    from concourse.bass_utils import run_bass_kernel_spmd

    z_ada = np.asarray(z_ada, np.float32)
    z_sou = np.asarray(z_sou, np.float32)
    in_maps = _make_in_maps(z_ada, z_sou)
    nc = _get_nc()
    res = run_bass_kernel_spmd(nc, in_maps, list(range(NCORES)))
    g_parts = [res.results[k]["G"] for k in range(NCORES)]
    return _host_finish(g_parts)


# revision 38
# speedup vs baseline: 1.0371x; 1.0205x over previous
"""Trainium2 kernel for nn_Loss_HF_86079734546730.

Strategy (8 NeuronCores, SPMD, no collectives):
  - Shard the two [64,3,512,512] inputs spatially over H: core k gets raw
    rows [64k, 64k+64) => shard [64, 3, 64, 512] per tensor (~25 MiB each).
  - DMA: 12 large HWDGE loads per core (one per tensor x channel x
    half-batch-group, ~4.2 MB each) issued on the sync queue -- amortizes
    the ~2us per-DMA completion latency that dominates with small DMAs.
  - Band build on PE only (no DVE preprocessing): per (tensor, channel,
    batch-pair j) tile [128=(2b x 64h), 512w] in fp32r, two matmuls per
    128-wb chunk accumulate the even-w and odd-w halves against constant
    Haar-combination matrices W_e / W_o [128, 256]. The stationary operand
    is the data (its free dim wb becomes the PSUM partition dim => output
    arrives transposed to [wb, feature-cols], exactly what the Gram stage
    needs), and the W columns encode the vertical Haar combos for all 3
    bands. fp32r runs at bf16 speed when the moving dim is >= 256 (W is
    zero-padded 192->256 for that reason).
  - ACT/DVE (alternating): copy PSUM [128, 0:192] -> bf16 band buffer,
    hb-major layout: col = hb*194 + (6j + b'*3 + band); col 192 of each
    194-block is a constant 1.0 (gives feature sums in the Gram).
  - PE: per (tensor, channel): Gram of the band buffer accumulated in PSUM
    fp32 over all 64 (wbc, hb) spatial chunks. Symmetric trick: chunk0
    computes rows 0:128 x cols 0:193, chunk1 only rows/cols 128:193; the
    host mirrors the missing block.
  - Host (float64): sum partial Grams over cores, rebuild per-(b,c,band)
    mean/std, expand the normalized-feature Gram algebraically, cosine-sim,
    softmax, KL.
"""

import numpy as np

B, C, H, W = 64, 3, 512, 512
NCORES = 8
HSH = H // NCORES          # 64 raw rows per core
NJ = B // 2                # 32 batch pairs
HB = HSH // 2              # 32 band rows per core
WB = W // 2                # 256 band cols
CCB = 194                  # band-buffer cols per hb block: 192 feat + 1 ones + 1 pad
EPS_STD = 1e-5
EPS_COS = 1e-8
EPS_P = 1e-8

_CACHE = {}


def _make_w():
    """Constant matrices for the band-build matmuls, [128, 512] fp32.

    Rows: p = (two in 2) x (h in 64)  [partition layout of the raw tiles]
    Cols: ew*256 + hb*6 + two'*3 + band   (band: 0=lh, 1=hh, 2=hl),
    cols 192:256 of each half are zero padding (keeps the fp32r moving dim
    at 256 for full rate).
      W[two*64 + 2*hb + eh, ew*256 + hb*6 + two*3 + band] = s(band, eh, ew)
      s(lh) = 0.5*(1-2*ew); s(hh) = -0.5 if eh==ew else 0.5; s(hl) = 0.5*(1-2*eh)
    """
    w = np.zeros((128, 512), np.float32)
    for two in range(2):
        for hb in range(HB):
            for eh in range(2):
                r = two * 64 + 2 * hb + eh
                for ew in range(2):
                    base = ew * 256 + hb * 6 + two * 3
                    w[r, base + 0] = 0.5 * (1 - 2 * ew)
                    w[r, base + 1] = -0.5 if eh == ew else 0.5
                    w[r, base + 2] = 0.5 * (1 - 2 * eh)
    return w


def _col_batch_map():
    """Global feature g = 6j + two'*3 + band  ->  batch 2j + two'."""
    cb = np.zeros(192, np.int64)
    for j in range(NJ):
        for bp in range(2):
            for band in range(3):
                cb[6 * j + bp * 3 + band] = 2 * j + bp
    return cb


def _build_nc():
    import concourse.bass as bass
    import concourse.mybir as mybir
    import concourse.tile as tile
    from concourse import bacc

    f32 = mybir.dt.float32
    f32r = mybir.dt.float32r
    bf16 = mybir.dt.bfloat16

    nc = bacc.Bacc()
    za = nc.declare_dram_parameter("za", [B, C, HSH, W], f32r, isOutput=False)
    zs = nc.declare_dram_parameter("zs", [B, C, HSH, W], f32r, isOutput=False)
    wmat = nc.declare_dram_parameter("wmat", [128, 512], f32r, isOutput=False)
    gout = nc.declare_dram_parameter("G", [2, C, 193, 193], f32, isOutput=True)
    zz = [za, zs]

    NBCOL = 2 * HB * CCB  # both wbc chunks in one buffer: 12416

    with tile.TileContext(nc) as tc:
        with (
            tc.tile_pool(name="wconst", bufs=1) as w_pool,
            tc.tile_pool(name="raw", bufs=6) as raw_pool,
            tc.tile_pool(name="bands", bufs=2) as band_pool,
            tc.tile_pool(name="stage", bufs=4) as stage_pool,
            tc.tile_pool(name="pband", bufs=6, space="PSUM") as pb_pool,
            tc.tile_pool(name="pgram", bufs=1, space="PSUM") as pg_pool,
        ):
            # W loads via SWDGE (gpsimd): putting it at the head of the sync
            # HWDGE ring would delay the first raw-tile load by the W DMA's
            # completion-receipt latency
            w_t = w_pool.tile([128, 512], f32r, tag="wmat")
            nc.gpsimd.dma_start(w_t[:], wmat[:])

            # the two load halves go through the two HWDGE rings (disjoint
            # even/odd SDMA engine sets -> they transfer concurrently)
            loaders = [nc.sync, nc.scalar]

            for c in range(C):
                bbs = {}
                for t in range(2):
                    bb = band_pool.tile([128, NBCOL], bf16, tag=f"bb{t}")
                    bb4 = bb[:].rearrange(
                        "p (wbc hb cc) -> p wbc hb cc", hb=HB, cc=CCB
                    )
                    nc.gpsimd.memset(bb4[:, :, :, 192], 1.0)
                    bbs[t] = bb

                for t in range(2):
                    # 8-j raw tiles with 6 buffers keep the load flow
                    # continuous (fine-grained buffer release) so the PE never
                    # starves at group boundaries.
                    for j0, j1 in [(0, 8), (8, 16), (16, 24), (24, 32)]:
                        nj = j1 - j0
                        raw = raw_pool.tile([128, nj * 512], f32r, tag="raw")
                        # DMA APs are limited to 3 dims, so split the load by
                        # the batch-within-pair dim; the two halves go to
                        # different HWDGE queues (disjoint even/odd SDMA
                        # engine sets) so they transfer concurrently. Both
                        # HWDGE engines (sync/scalar) carry nothing else, so
                        # ring blocking never stalls compute issue.
                        src = zz[t][2 * j0 : 2 * j1, c].rearrange(
                            "(j two) h w -> two h j w", two=2
                        )
                        for two in range(2):
                            loaders[two].dma_start(
                                raw[64 * two : 64 * (two + 1), :], src[two]
                            )
                        for jl in range(nj):
                            j = j0 + jl
                            rw = raw[:, 512 * jl : 512 * (jl + 1)].rearrange(
                                "p (w two) -> p w two", two=2
                            )
                            # both wbc chunks in one PSUM bank tile; the two
                            # accumulation groups are strictly ordered so the
                            # bank-wide has_written clear of the second group
                            # cannot corrupt the first
                            # both wbc chunks in one PSUM bank tile; the two
                            # accumulation groups are strictly ordered so the
                            # bank-wide has_written clear of the second group
                            # cannot corrupt the first
                            pband = pb_pool.tile([128, 512], f32, tag="pband")
                            for wbc in range(2):
                                for ew in range(2):
                                    nc.tensor.matmul(
                                        pband[:, 256 * wbc : 256 * (wbc + 1)],
                                        rw[:, 128 * wbc : 128 * (wbc + 1), ew],
                                        w_t[:, 256 * ew : 256 * ew + 256],
                                        start=(ew == 0),
                                        stop=(ew == 1),
                                    )
                            # single fused DVE copy (PSUM fp32 -> bf16 bands).
                            # ACT must NOT take copies: it carries a HWDGE
                            # load ring, and ring-blocked DMA instructions
                            # would stall queued ACT copies (measured 25%+
                            # regression when tried).
                            src_v = pband[:].rearrange(
                                "p (wbc x) -> p wbc x", x=256
                            )[:, :, 0:192].rearrange(
                                "p wbc (hb l) -> p wbc hb l", l=6
                            )
                            dst_v = bbs[t][:].rearrange(
                                "p (wbc hb cc) -> p wbc hb cc", hb=HB, cc=CCB
                            )[:, :, :, 6 * j : 6 * j + 6]
                            nc.vector.tensor_copy(dst_v, src_v)

                    # gram(t) right after bands(t): fills the PE while the
                    # next group's loads arrive, and shortens the end tail
                    bb4 = bbs[t][:].rearrange(
                        "p (wbc hb cc) -> p wbc hb cc", hb=HB, cc=CCB
                    )
                    # chunk0: rows 0:128 x cols 0:193; chunk1: rows/cols 128:193
                    for chunk in range(2):
                        if chunk == 0:
                            rows, cs, ce, ms, me = 128, 0, 128, 0, 193
                        else:
                            rows, cs, ce, ms, me = 65, 128, 193, 128, 193
                        pg = pg_pool.tile([rows, me - ms], f32, tag=f"pg{chunk}")
                        for wbc in range(2):
                            for hb in range(HB):
                                nc.tensor.matmul(
                                    pg[:],
                                    bb4[:, wbc, hb, cs:ce],
                                    bb4[:, wbc, hb, ms:me],
                                    start=(wbc == 0 and hb == 0),
                                    stop=(wbc == 1 and hb == HB - 1),
                                )
                        st = stage_pool.tile([rows, me - ms], f32, tag=f"st{chunk}")
                        nc.vector.tensor_copy(st[:], pg[:])
                        # store via SWDGE (gpsimd is otherwise idle) so it
                        # never stalls the two HWDGE load rings
                        nc.gpsimd.dma_start(
                            gout[t, c, cs : cs + rows, ms:me], st[:]
                        )
    if not nc.is_finalized():
        nc.finalize()
    return nc


def _get_nc():
    if "nc" not in _CACHE:
        _CACHE["nc"] = _build_nc()
    return _CACHE["nc"]


def _host_finish(g_parts):
    """g_parts: list of per-core G arrays [2,3,193,193] (fp32). Returns KL."""
    g = np.zeros((2, C, 193, 193), np.float64)
    for arr in g_parts:
        g += np.asarray(arr, np.float64)
    # mirror the symmetric block the kernel skipped
    g[:, :, 128:, :128] = np.swapaxes(g[:, :, :128, 128:], -1, -2)

    cb = _col_batch_map()
    S = float(g[0, 0, 192, 192])

    P = np.zeros((2, B, B), np.float64)
    Bm = np.zeros((192, B), np.float64)
    Bm[np.arange(192), cb] = 1.0
    for t in range(2):
        for c in range(C):
            M = g[t, c, :192, :192]
            Tv = g[t, c, 192, :192]
            mu = Tv / S
            var = (np.diag(M) - Tv * Tv / S) / (S - 1.0)
            sig = np.sqrt(np.maximum(var, 0.0))
            alpha = 1.0 / (3.0 * (sig + EPS_STD))
            Mc = M - np.outer(mu, Tv) - np.outer(Tv, mu) + S * np.outer(mu, mu)
            Ms = (alpha[:, None] * Mc) * alpha[None, :]
            P[t] += Bm.T @ Ms @ Bm

    sims = []
    for t in range(2):
        r = np.sqrt(np.maximum(np.diag(P[t]), 0.0))
        rc = np.maximum(r, EPS_COS)
        sims.append(P[t] / np.outer(rc, rc))

    def softmax_offdiag(sim):
        m = sim.copy()
        np.fill_diagonal(m, -np.inf)
        mx = m.max(axis=1, keepdims=True)
        e = np.exp(m - mx)
        return e / e.sum(axis=1, keepdims=True)

    p_ada = softmax_offdiag(sims[0]) + EPS_P
    p_sou = softmax_offdiag(sims[1]) + EPS_P
    kl = np.sum(p_sou * (np.log(p_sou) - np.log(p_ada))) / B
    return np.float32(kl)


def _make_in_maps(z_ada, z_sou):
    wmat = _make_w()
    in_maps = []
    for k in range(NCORES):
        sl = slice(HSH * k, HSH * (k + 1))
        in_maps.append(
            {
                "za": np.ascontiguousarray(z_ada[:, :, sl, :]),
                "zs": np.ascontiguousarray(z_sou[:, :, sl, :]),
                "wmat": wmat,
            }
        )
    return in_maps


def kernel(z_ada, z_sou):
    from concourse.bass_utils import run_bass_kernel_spmd

    z_ada = np.asarray(z_ada, np.float32)
    z_sou = np.asarray(z_sou, np.float32)
    in_maps = _make_in_maps(z_ada, z_sou)
    nc = _get_nc()
    res = run_bass_kernel_spmd(nc, in_maps, list(range(NCORES)))
    g_parts = [res.results[k]["G"] for k in range(NCORES)]
    return _host_finish(g_parts)


# revision 40
# speedup vs baseline: 1.0451x; 1.0077x over previous
"""Trainium2 kernel for nn_Loss_HF_86079734546730.

Strategy (8 NeuronCores, SPMD, no collectives):
  - Shard the two [64,3,512,512] inputs spatially over H: core k gets raw
    rows [64k, 64k+64) => shard [64, 3, 64, 512] per tensor (~25 MiB each).
  - DMA: 12 large HWDGE loads per core (one per tensor x channel x
    half-batch-group, ~4.2 MB each) issued on the sync queue -- amortizes
    the ~2us per-DMA completion latency that dominates with small DMAs.
  - Band build on PE only (no DVE preprocessing): per (tensor, channel,
    batch-pair j) tile [128=(2b x 64h), 512w] in fp32r, two matmuls per
    128-wb chunk accumulate the even-w and odd-w halves against constant
    Haar-combination matrices W_e / W_o [128, 256]. The stationary operand
    is the data (its free dim wb becomes the PSUM partition dim => output
    arrives transposed to [wb, feature-cols], exactly what the Gram stage
    needs), and the W columns encode the vertical Haar combos for all 3
    bands. fp32r runs at bf16 speed when the moving dim is >= 256 (W is
    zero-padded 192->256 for that reason).
  - ACT/DVE (alternating): copy PSUM [128, 0:192] -> bf16 band buffer,
    hb-major layout: col = hb*194 + (6j + b'*3 + band); col 192 of each
    194-block is a constant 1.0 (gives feature sums in the Gram).
  - PE: per (tensor, channel): Gram of the band buffer accumulated in PSUM
    fp32 over all 64 (wbc, hb) spatial chunks. Symmetric trick: chunk0
    computes rows 0:128 x cols 0:193, chunk1 only rows/cols 128:193; the
    host mirrors the missing block.
  - Host (float64): sum partial Grams over cores, rebuild per-(b,c,band)
    mean/std, expand the normalized-feature Gram algebraically, cosine-sim,
    softmax, KL.
"""

import numpy as np

B, C, H, W = 64, 3, 512, 512
NCORES = 8
HSH = H // NCORES          # 64 raw rows per core
NJ = B // 2                # 32 batch pairs
HB = HSH // 2              # 32 band rows per core
WB = W // 2                # 256 band cols
CCB = 194                  # band-buffer cols per hb block: 192 feat + 1 ones + 1 pad
EPS_STD = 1e-5
EPS_COS = 1e-8
EPS_P = 1e-8

_CACHE = {}


def _make_w():
    """Constant matrices for the band-build matmuls, [128, 512] fp32.

    Rows: p = (two in 2) x (h in 64)  [partition layout of the raw tiles]
    Cols: ew*256 + hb*6 + two'*3 + band   (band: 0=lh, 1=hh, 2=hl),
    cols 192:256 of each half are zero padding (keeps the fp32r moving dim
    at 256 for full rate).
      W[two*64 + 2*hb + eh, ew*256 + hb*6 + two*3 + band] = s(band, eh, ew)
      s(lh) = 0.5*(1-2*ew); s(hh) = -0.5 if eh==ew else 0.5; s(hl) = 0.5*(1-2*eh)
    """
    w = np.zeros((128, 512), np.float32)
    for two in range(2):
        for hb in range(HB):
            for eh in range(2):
                r = two * 64 + 2 * hb + eh
                for ew in range(2):
                    base = ew * 256 + hb * 6 + two * 3
                    w[r, base + 0] = 0.5 * (1 - 2 * ew)
                    w[r, base + 1] = -0.5 if eh == ew else 0.5
                    w[r, base + 2] = 0.5 * (1 - 2 * eh)
    return w


def _col_batch_map():
    """Global feature g = 6j + two'*3 + band  ->  batch 2j + two'."""
    cb = np.zeros(192, np.int64)
    for j in range(NJ):
        for bp in range(2):
            for band in range(3):
                cb[6 * j + bp * 3 + band] = 2 * j + bp
    return cb


def _build_nc():
    import concourse.bass as bass
    import concourse.mybir as mybir
    import concourse.tile as tile
    from concourse import bacc

    f32 = mybir.dt.float32
    f32r = mybir.dt.float32r
    bf16 = mybir.dt.bfloat16

    nc = bacc.Bacc()
    za = nc.declare_dram_parameter("za", [B, C, HSH, W], f32r, isOutput=False)
    zs = nc.declare_dram_parameter("zs", [B, C, HSH, W], f32r, isOutput=False)
    wmat = nc.declare_dram_parameter("wmat", [128, 512], f32r, isOutput=False)
    gout = nc.declare_dram_parameter("G", [2, C, 193, 193], f32, isOutput=True)
    zz = [za, zs]

    NBCOL = 2 * HB * CCB  # both wbc chunks in one buffer: 12416

    with tile.TileContext(nc) as tc:
        with (
            tc.tile_pool(name="wconst", bufs=1) as w_pool,
            tc.tile_pool(name="raw", bufs=6) as raw_pool,
            tc.tile_pool(name="bands", bufs=2) as band_pool,
            tc.tile_pool(name="stage", bufs=8) as stage_pool,
            tc.tile_pool(name="pband", bufs=6, space="PSUM") as pb_pool,
            tc.tile_pool(name="pgram", bufs=1, space="PSUM") as pg_pool,
        ):
            w_t = w_pool.tile([128, 512], f32r, tag="wmat")
            nc.sync.dma_start(w_t[:], wmat[:])

            # the two load halves go through the two HWDGE rings (disjoint
            # even/odd SDMA engine sets -> they transfer concurrently)
            loaders = [nc.sync, nc.scalar]

            for c in range(C):
                bbs = {}
                for t in range(2):
                    bb = band_pool.tile([128, NBCOL], bf16, tag=f"bb{t}")
                    bb4 = bb[:].rearrange(
                        "p (wbc hb cc) -> p wbc hb cc", hb=HB, cc=CCB
                    )
                    nc.gpsimd.memset(bb4[:, :, :, 192], 1.0)
                    bbs[t] = bb

                for t in range(2):
                    # 8-j raw tiles with 6 buffers keep the load flow
                    # continuous (fine-grained buffer release) so the PE never
                    # starves at group boundaries.
                    for j0, j1 in [(0, 8), (8, 16), (16, 24), (24, 32)]:
                        nj = j1 - j0
                        raw = raw_pool.tile([128, nj * 512], f32r, tag="raw")
                        # DMA APs are limited to 3 dims, so split the load by
                        # the batch-within-pair dim; the two halves go to
                        # different HWDGE queues (disjoint even/odd SDMA
                        # engine sets) so they transfer concurrently. Both
                        # HWDGE engines (sync/scalar) carry nothing else, so
                        # ring blocking never stalls compute issue.
                        src = zz[t][2 * j0 : 2 * j1, c].rearrange(
                            "(j two) h w -> two h j w", two=2
                        )
                        for two in range(2):
                            loaders[two].dma_start(
                                raw[64 * two : 64 * (two + 1), :], src[two]
                            )
                        for jl in range(nj):
                            j = j0 + jl
                            rw = raw[:, 512 * jl : 512 * (jl + 1)].rearrange(
                                "p (w two) -> p w two", two=2
                            )
                            # both wbc chunks in one PSUM bank tile; the two
                            # accumulation groups are strictly ordered so the
                            # bank-wide has_written clear of the second group
                            # cannot corrupt the first
                            # both wbc chunks in one PSUM bank tile; the two
                            # accumulation groups are strictly ordered so the
                            # bank-wide has_written clear of the second group
                            # cannot corrupt the first
                            pband = pb_pool.tile([128, 512], f32, tag="pband")
                            for wbc in range(2):
                                for ew in range(2):
                                    nc.tensor.matmul(
                                        pband[:, 256 * wbc : 256 * (wbc + 1)],
                                        rw[:, 128 * wbc : 128 * (wbc + 1), ew],
                                        w_t[:, 256 * ew : 256 * ew + 256],
                                        start=(ew == 0),
                                        stop=(ew == 1),
                                    )
                            # single fused DVE copy (PSUM fp32 -> bf16 bands).
                            # ACT must NOT take copies: it carries a HWDGE
                            # load ring, and ring-blocked DMA instructions
                            # would stall queued ACT copies (measured 25%+
                            # regression when tried).
                            src_v = pband[:].rearrange(
                                "p (wbc x) -> p wbc x", x=256
                            )[:, :, 0:192].rearrange(
                                "p wbc (hb l) -> p wbc hb l", l=6
                            )
                            dst_v = bbs[t][:].rearrange(
                                "p (wbc hb cc) -> p wbc hb cc", hb=HB, cc=CCB
                            )[:, :, :, 6 * j : 6 * j + 6]
                            nc.vector.tensor_copy(dst_v, src_v)

                    # gram(t) right after bands(t): fills the PE while the
                    # next group's loads arrive, and shortens the end tail
                    bb4 = bbs[t][:].rearrange(
                        "p (wbc hb cc) -> p wbc hb cc", hb=HB, cc=CCB
                    )
                    # chunk0: rows 0:128 x cols 0:193; chunk1: rows/cols 128:193
                    for chunk in range(2):
                        if chunk == 0:
                            rows, cs, ce, ms, me = 128, 0, 128, 0, 193
                        else:
                            rows, cs, ce, ms, me = 65, 128, 193, 128, 193
                        pg = pg_pool.tile([rows, me - ms], f32, tag=f"pg{chunk}")
                        for wbc in range(2):
                            for hb in range(HB):
                                nc.tensor.matmul(
                                    pg[:],
                                    bb4[:, wbc, hb, cs:ce],
                                    bb4[:, wbc, hb, ms:me],
                                    start=(wbc == 0 and hb == 0),
                                    stop=(wbc == 1 and hb == HB - 1),
                                )
                        st = stage_pool.tile([rows, me - ms], f32, tag=f"st{chunk}")
                        nc.vector.tensor_copy(st[:], pg[:])
                        # store via SWDGE (gpsimd is otherwise idle) so it
                        # never stalls the two HWDGE load rings
                        nc.gpsimd.dma_start(
                            gout[t, c, cs : cs + rows, ms:me], st[:]
                        )
    if not nc.is_finalized():
        nc.finalize()
    return nc


def _get_nc():
    if "nc" not in _CACHE:
        _CACHE["nc"] = _build_nc()
    return _CACHE["nc"]


def _host_finish(g_parts):
    """g_parts: list of per-core G arrays [2,3,193,193] (fp32). Returns KL."""
    g = np.zeros((2, C, 193, 193), np.float64)
    for arr in g_parts:
        g += np.asarray(arr, np.float64)
    # mirror the symmetric block the kernel skipped
    g[:, :, 128:, :128] = np.swapaxes(g[:, :, :128, 128:], -1, -2)

    cb = _col_batch_map()
    S = float(g[0, 0, 192, 192])

    P = np.zeros((2, B, B), np.float64)
    Bm = np.zeros((192, B), np.float64)
    Bm[np.arange(192), cb] = 1.0
    for t in range(2):
        for c in range(C):
            M = g[t, c, :192, :192]
            Tv = g[t, c, 192, :192]
            mu = Tv / S
            var = (np.diag(M) - Tv * Tv / S) / (S - 1.0)
            sig = np.sqrt(np.maximum(var, 0.0))
            alpha = 1.0 / (3.0 * (sig + EPS_STD))
            Mc = M - np.outer(mu, Tv) - np.outer(Tv, mu) + S * np.outer(mu, mu)
            Ms = (alpha[:, None] * Mc) * alpha[None, :]
            P[t] += Bm.T @ Ms @ Bm

    sims = []
    for t in range(2):
        r = np.sqrt(np.maximum(np.diag(P[t]), 0.0))
        rc = np.maximum(r, EPS_COS)
        sims.append(P[t] / np.outer(rc, rc))

    def softmax_offdiag(sim):
        m = sim.copy()
        np.fill_diagonal(m, -np.inf)
        mx = m.max(axis=1, keepdims=True)
        e = np.exp(m - mx)
        return e / e.sum(axis=1, keepdims=True)

    p_ada = softmax_offdiag(sims[0]) + EPS_P
    p_sou = softmax_offdiag(sims[1]) + EPS_P
    kl = np.sum(p_sou * (np.log(p_sou) - np.log(p_ada))) / B
    return np.float32(kl)


def _make_in_maps(z_ada, z_sou):
    wmat = _make_w()
    in_maps = []
    for k in range(NCORES):
        sl = slice(HSH * k, HSH * (k + 1))
        in_maps.append(
            {
                "za": np.ascontiguousarray(z_ada[:, :, sl, :]),
                "zs": np.ascontiguousarray(z_sou[:, :, sl, :]),
                "wmat": wmat,
            }
        )
    return in_maps


def kernel(z_ada, z_sou):
    from concourse.bass_utils import run_bass_kernel_spmd

    z_ada = np.asarray(z_ada, np.float32)
    z_sou = np.asarray(z_sou, np.float32)
    in_maps = _make_in_maps(z_ada, z_sou)
    nc = _get_nc()
    res = run_bass_kernel_spmd(nc, in_maps, list(range(NCORES)))
    g_parts = [res.results[k]["G"] for k in range(NCORES)]
    return _host_finish(g_parts)
